# revision 1
# baseline (speedup 1.0000x reference)
"""Chamfer 1D loss on 8 TRN2 NeuronCores.

Sharding: core c owns x[2048c:2048(c+1)] and y[2048c:2048(c+1)] as "row"
blocks; each direction's min is computed against the FULL other array
(replicated to every core), so no inter-core collective is needed — each
core emits partial row-minima and the host sums them.

Per core, per direction: rows [2048] live one-per-partition-lane as 16
tiles of [128,1]; the full opposing array [16384] is partition-broadcast
into SBUF in chunks; one fused custom DVE instruction per (chunk, tile)
computes |row - col| and min-reduces it (exact fp32), writing one cell of
a [128, 16*n_chunks] partial-min matrix. A final tensor_reduce(min)
collapses chunks. Output per core: [2, 128, 16] row minima.
"""

import numpy as np

import concourse.bacc as bacc
import concourse.mybir as mybir
import concourse.tile as tile
import concourse.bass_utils as bass_utils

import concourse.bass_isa as bass_isa
import concourse.dve_ops as dve_ops
from concourse.dve_ops import DveOp, get_dve_sub_opcode
from concourse.dve_spec import Spec, Src0, C0, C1, Zero, maxx, minn, lower, _has_src1
from concourse.dve_uop import DveOpSpec

F32 = mybir.dt.float32
P = 128          # partitions
NF = 16384       # full length of each input
NB = NF // 8     # row block per core (2048)
T = NB // P      # row tiles per block (16)
# Direction 0 streams the opposing array in small chunks so the first
# custom op starts ~3us in; direction 1 uses one big chunk whose
# broadcast DMA fully overlaps direction-0 compute.
CHUNKS = [[1536, 14848], [16384]]
ALPHA = 0.5
BIG = 3.0e38

OP_NAME = "CHAMFER_ABSDIFF_MIN_ANT"


def _ref(in0, in1, s0, s1, imm2):
    x = in0.astype(np.float32)
    body = np.abs(x - s0).astype(np.float32)
    seed = s1 if isinstance(s1, (int, float)) else np.asarray(s1, np.float32)
    flat = body.reshape(body.shape[0], -1)
    acc = np.minimum(flat.min(axis=-1, keepdims=True), seed)
    if isinstance(acc, np.ndarray):
        acc = acc.reshape(body.shape[0], 1)
    return body, acc


def _register_op() -> DveOp:
    if OP_NAME in dve_ops._SUB_OPCODE_FOR_NAME:
        for op in dve_ops.OPS:
            if op.name == OP_NAME:
                return op
        raise RuntimeError("row allocated but op missing")
    d = Src0 - C0
    spec = Spec(body=maxx(d, Zero - d), accum=minn, accum_init=C1, reference=_ref)
    row = dve_ops._CUSTOM_DVE_ROW_BASE + len(dve_ops.OPS)
    assert row < 0x20
    dve_ops._SUB_OPCODE_FOR_NAME[OP_NAME] = row
    shas = {}
    for ver in ("v3", "v4"):
        shas[ver] = DveOpSpec(
            name=OP_NAME, opcode=row, uops=lower(spec, ver=ver),
            rd1_en=_has_src1(spec),
        ).sha(ver)
    op = DveOp(OP_NAME, spec, subdim=False, uops_sha=shas)
    dve_ops.OPS.append(op)
    dve_ops.CUSTOM_DVE_SPECS[OP_NAME] = spec
    return op


CHAMFER_OP = _register_op()


def _emit_chamfer(vec, *, out, in0, s0, s1, accum_out):
    """_custom_dve replica that skips AP optimization on `out` so a step-0
    broadcast write pattern (scratch-free body sink) survives lowering."""
    op = CHAMFER_OP
    bassm = vec.bass
    if op.name not in bassm.m.ant_custom_dve_ops:
        bassm.m.ant_custom_dve_ops = sorted({*bassm.m.ant_custom_dve_ops, op.name})
    op.compile("v3")
    shape = bass_isa.CustomDveShape.TTSS
    opc = bassm.isa.Opcode[
        f"NEURON_ISA_TPB_OPCODE_CUSTOM_DVE_ANT_{shape.slot()}"
    ].value

    def lower_scalar(v):
        if isinstance(v, (int, float)):
            return mybir.ImmediateValue(dtype=mybir.dt.float32, value=float(v))
        return vec.lower_ap(v, for_isa=True)

    ins_l = [
        vec.lower_ap(in0, for_isa=True, opt=True),
        lower_scalar(s0),
        lower_scalar(s1),
    ]
    outs_l = [
        vec.lower_ap(out, for_isa=True, opt=False),
        vec.lower_ap(accum_out, for_isa=True),
    ]
    return vec.add_instruction(
        bass_isa.InstCustomDveAnt(
            name=bassm.get_next_instruction_name(),
            op_name=op.name,
            rd1_en=False,
            subdim=0,
            imm2=0.0,
            shape=shape,
            row=get_dve_sub_opcode(op.name),
            isa_opcode=opc,
            ins=ins_l,
            outs=outs_l,
        )
    )


_NC_CACHE = None


def _build():
    global _NC_CACHE
    if _NC_CACHE is not None:
        return _NC_CACHE
    nc = bacc.Bacc("TRN2", target_bir_lowering=False, debug=False, num_devices=8)
    x_blk = nc.dram_tensor("x_blk", [NB], F32, kind="ExternalInput")
    y_blk = nc.dram_tensor("y_blk", [NB], F32, kind="ExternalInput")
    x_full = nc.dram_tensor("x_full", [NF], F32, kind="ExternalInput")
    y_full = nc.dram_tensor("y_full", [NF], F32, kind="ExternalInput")
    mins = nc.dram_tensor("mins", [2, P, T], F32, kind="ExternalOutput")

    with tile.TileContext(nc) as tc:
        with (
            tc.tile_pool(name="bcast0", bufs=1) as bc_pool0,
            tc.tile_pool(name="bcast1", bufs=1) as bc_pool1,
            tc.tile_pool(name="scratch", bufs=2) as sc_pool,
            tc.tile_pool(name="small", bufs=1) as small,
        ):
            for d, (rows_dram, cols_dram) in enumerate(
                [(x_blk, y_full), (y_blk, x_full)]
            ):
                chunks = CHUNKS[d]
                nch = len(chunks)
                bc_pool = bc_pool0 if d == 0 else bc_pool1
                # rows: [128, T], partition p / tile t <- rows_dram[p*T + t]
                rows_sb = small.tile([P, T], F32, tag=f"rows{d}")
                # tiny strided row loads ride the ACT ring so they don't
                # serialize ahead of the first broadcast on the SP ring
                nc.scalar.dma_start(
                    rows_sb[:], rows_dram.ap().rearrange("(p t) -> p t", p=P)
                )
                # row minima, carry-chained across chunks via the accum seed
                minw = small.tile([P, T], F32, tag=f"minw{d}")
                # pre-issue every broadcast DMA (all on the SP HWDGE ring;
                # the ACT ring measured ~20% slower for these broadcasts)
                cols_tiles = []
                off = 0
                for ch, cw in enumerate(chunks):
                    cols_sb = bc_pool.tile([P, cw], F32, tag=f"cols{d}c{ch}")
                    nc.sync.dma_start(
                        cols_sb[:],
                        cols_dram.ap()[off : off + cw]
                        .unsqueeze(0)
                        .partition_broadcast(P),
                    )
                    cols_tiles.append(cols_sb)
                    off += cw
                for ch, cw in enumerate(chunks):
                    cols_sb = cols_tiles[ch]
                    for t in range(T):
                        # body values are scratch: sink every write into one
                        # cell via a step-0 broadcast AP (no big scratch tile)
                        scr = sc_pool.tile([P, 1], F32, tag="scr")
                        _emit_chamfer(
                            nc.vector,
                            out=scr[:, 0:1].to_broadcast([P, cw]),
                            in0=cols_sb[:],
                            s0=rows_sb[:, t : t + 1],
                            s1=BIG if ch == 0 else minw[:, t : t + 1],
                            accum_out=minw[:, t : t + 1],
                        )
                nc.sync.dma_start(mins.ap()[d], minw[:])
    nc.compile()
    _NC_CACHE = nc
    return nc


def kernel(**inputs: np.ndarray) -> np.ndarray:
    x = np.ascontiguousarray(inputs["inputs"], dtype=np.float32).reshape(-1)
    y = np.ascontiguousarray(inputs["targets"], dtype=np.float32).reshape(-1)
    assert x.shape == (NF,) and y.shape == (NF,)

    nc = _build()
    in_maps = [
        {
            "x_blk": x[c * NB : (c + 1) * NB],
            "y_blk": y[c * NB : (c + 1) * NB],
            "x_full": x,
            "y_full": y,
        }
        for c in range(8)
    ]
    res = bass_utils.run_bass_kernel_spmd(nc, in_maps, core_ids=list(range(8)))

    cd_xy = 0.0
    cd_yx = 0.0
    for c in range(8):
        m = res.results[c]["mins"]
        cd_xy += m[0].sum(dtype=np.float64)
        cd_yx += m[1].sum(dtype=np.float64)
    val = ALPHA * cd_xy / NF + (1.0 - ALPHA) * cd_yx / NF
    return np.float32(val)



# revision 2
# speedup vs baseline: 1.8750x; 1.8750x over previous
"""Chamfer 1D loss on 8 TRN2 NeuronCores — dual-tile squared-distance kernel.

Sharding: core c owns x[2048c:2048(c+1)] and y[2048c:2048(c+1)] as "row"
blocks; each direction's min is computed against the FULL other array
(replicated to every core), so no inter-core collective is needed — each
core emits per-row minima of SQUARED distances and the host takes
sqrt + sums (min_j |d| = sqrt(min_j d^2), exact up to fp rounding).

Per core, per direction: 2048 rows live one-per-partition-lane as 16
tiles of [128,1]; the full opposing array [16384] is partition-broadcast
into SBUF. One custom "dual-tile" DVE instruction per PAIR of row tiles
streams a column chunk once and computes BOTH tiles' running minima:

  s0: d_t = y - x_t     s3: d_u = y - x_u
  s1: d_t^2             s4: d_u^2
  s2: acc_t = min(..)   s5: acc_u = min(..)

i.e. 2 row-column pairs per element read per cycle — 2x the throughput
of the 1-elem/cycle fused op (the DVE's 2-read-port ceiling, reached
here with one port in plain REGULAR mode). The two accumulators live in
stage-2/stage-5 CURR_ALU_OUT flops, seeded from imm2 by a 1-count seed
uop and drained after SRC_TENSOR_DONE by two 1-count writer uops (the
stock FIND_INDEX_8 post-stream pattern).

Output per core: mins0 [128,16,2] (x-rows, 2 column chunks) and
mins1 [128,16] (y-rows, 1 chunk) of squared minima.
"""

import numpy as np

import concourse.bacc as bacc
import concourse.mybir as mybir
import concourse.tile as tile
import concourse.bass_utils as bass_utils

import concourse.bass_isa as bass_isa
import concourse.dve_ops as dve_ops
from concourse.dve_ops import get_dve_sub_opcode
from concourse.dve_spec import Spec, Src0, C0, C1, minn, sq
from concourse.dve_uop import (
    AluInp,
    AluOp,
    DelayInp,
    DveOpSpec,
    InpSel,
    OutPath,
    OutSel,
    Trigger,
    UopConfig,
)

F32 = mybir.dt.float32
P = 128          # partitions
NF = 16384       # full length of each input
NB = NF // 8     # row block per core (2048)
T = NB // P      # row tiles per block (16)
NPAIR = T // 2   # dual-tile instructions per (direction, chunk)
# Direction 0 streams the opposing array in two chunks so the first
# instruction starts as soon as ~2048 columns have broadcast; direction
# 1 uses one chunk whose broadcast fully overlaps direction-0 compute.
CHUNKS0 = [2048, 14336]
ALPHA = 0.5
BIG = 3.0e38

OP_NAME = "CHAMFER_SQD2_ANT"
_D = AluInp


def _dual_uops() -> list[UopConfig]:
    """[seed, steady, spacer, drain_t, drain_u].

    Lane map: D0=Src0 (column value), D1=CONST_0 (x_t), D2=CONST_1 (x_u),
    D3=CONST_2 (imm2 accumulator seed).
    """

    def route(u: UopConfig) -> UopConfig:
        u.enable_input(InpSel.SRC_0, 1)
        u.enable_input(InpSel.CONST_0, 2)
        u.enable_input(InpSel.CONST_1, 3)
        u.enable_input(InpSel.CONST_2, 4)
        return u

    # seed: CURR[s2] <- imm2, CURR[s5] <- imm2 (one bubble element)
    u0 = route(UopConfig())
    dp = u0.datapath_config
    for s in (0, 1):
        dp[s].pass_through_delay(3)
    dp[2].enable_alu(AluOp.BYPASS, _D.PREV_DELAY_3).pass_through_delay(3)
    for s in (3, 4):
        dp[s].pass_through_delay(3)
    dp[5].enable_alu(AluOp.BYPASS, _D.PREV_DELAY_3)
    u0.repeat_count = 1
    u0.trigger = (Trigger.COUNT, Trigger.NONE, Trigger.NONE)
    u0.next_uop = (1, 0, 0)

    # steady: both chains, one element consumed per cycle
    u1 = route(UopConfig())
    dp = u1.datapath_config
    dp[0].enable_alu(AluOp.SUBTRACT, _D.PREV_DELAY_0, _D.PREV_DELAY_1)
    dp[0].pass_through_delay(0, 2)
    dp[1].enable_alu(AluOp.MULTIPLY, _D.PREV_ALU_OUT, _D.PREV_ALU_OUT)
    dp[1].pass_through_delay(0, 2)
    dp[2].enable_alu(AluOp.MIN, _D.CURR_ALU_OUT, _D.PREV_ALU_OUT)
    dp[2].pass_through_delay(0, 2)
    dp[3].enable_alu(AluOp.SUBTRACT, _D.PREV_DELAY_0, _D.PREV_DELAY_2)
    dp[4].enable_alu(AluOp.MULTIPLY, _D.PREV_ALU_OUT, _D.PREV_ALU_OUT)
    dp[5].enable_alu(AluOp.MIN, _D.CURR_ALU_OUT, _D.PREV_ALU_OUT)
    u1.require_inp0 = 1
    u1.trigger = (Trigger.SRC_TENSOR_DONE, Trigger.NONE, Trigger.NONE)
    u1.next_uop = (2, 0, 0)

    # spacer: pure bubble (touches no flops) so the last real element
    # clears the accumulator stages before the drain elements read them
    u2 = route(UopConfig())
    u2.repeat_count = 1
    u2.trigger = (Trigger.COUNT, Trigger.NONE, Trigger.NONE)
    u2.next_uop = (3, 0, 0)

    # drain_t: emit CURR[s2] via delay lane 0 (s5's flop holds acc_u and
    # must not be written, so the value bypasses the ALU chain)
    u3 = route(UopConfig())
    dp = u3.datapath_config
    dp[2].enable_alu(AluOp.BYPASS, _D.CURR_ALU_OUT)
    dp[3].enable_delay_from_src(DelayInp.PREV_ALU_OUT, 0)
    for s in (4, 5, 6, 7):
        dp[s].pass_through_delay(0)
    u3.enable_output(OutSel.DELAY_0, OutPath.WR0_LO)
    u3.repeat_count = 1
    u3.trigger = (Trigger.COUNT, Trigger.NONE, Trigger.NONE)
    u3.next_uop = (4, 0, 0)

    # drain_u: emit CURR[s5] via the ALU chain
    u4 = route(UopConfig())
    dp = u4.datapath_config
    dp[5].enable_alu(AluOp.BYPASS, _D.CURR_ALU_OUT)
    dp[6].pass_through_alu()
    dp[7].pass_through_alu()
    u4.enable_output(OutSel.ALU_OUT, OutPath.WR0_LO)
    u4.repeat_count = 1
    u4.trigger = (Trigger.COUNT, Trigger.NONE, Trigger.NONE)
    u4.next_uop = (0, 0, 0)

    return [u0, u1, u2, u3, u4]


class _DualOp:
    """Duck-typed dve_ops.DveOp with a hand-written uop chain."""

    def __init__(self, name: str, spec: Spec):
        self.name = name
        self.spec = spec
        self.subdim = False
        self._cache: dict[str, DveOpSpec] = {}

    def compile(self, ver: str) -> DveOpSpec:
        if ver in self._cache:
            return self._cache[ver]
        assert ver == "v3", "kernel targets TRN2"
        s = DveOpSpec(
            name=self.name,
            opcode=get_dve_sub_opcode(self.name),
            uops=_dual_uops(),
            rd1_en=False,
        )
        self._cache[ver] = s
        return s


def _register() -> _DualOp:
    if OP_NAME in dve_ops._SUB_OPCODE_FOR_NAME:
        for op in dve_ops.OPS:
            if op.name == OP_NAME:
                return op
        raise RuntimeError("row allocated but op missing")
    # registry-compat spec (sims only; HW semantics come from _dual_uops)
    spec = Spec(body=sq(Src0 - C0), accum=minn, accum_init=C1)
    row = dve_ops._CUSTOM_DVE_ROW_BASE + len(dve_ops.OPS)
    assert row < 0x20
    dve_ops._SUB_OPCODE_FOR_NAME[OP_NAME] = row
    op = _DualOp(OP_NAME, spec)
    dve_ops.OPS.append(op)
    dve_ops.CUSTOM_DVE_SPECS[OP_NAME] = spec
    return op


SQD2 = _register()


def _emit(vec, *, out, in0, s0, s1):
    op = SQD2
    bassm = vec.bass
    if op.name not in bassm.m.ant_custom_dve_ops:
        bassm.m.ant_custom_dve_ops = sorted({*bassm.m.ant_custom_dve_ops, op.name})
    op.compile("v3")
    shape = bass_isa.CustomDveShape.TTSS
    opc = bassm.isa.Opcode[
        f"NEURON_ISA_TPB_OPCODE_CUSTOM_DVE_ANT_{shape.slot()}"
    ].value
    ins_l = [
        vec.lower_ap(in0, for_isa=True, opt=True),
        vec.lower_ap(s0, for_isa=True),
        vec.lower_ap(s1, for_isa=True),
    ]
    outs_l = [vec.lower_ap(out, for_isa=True)]
    return vec.add_instruction(
        bass_isa.InstCustomDveAnt(
            name=bassm.get_next_instruction_name(),
            op_name=op.name,
            rd1_en=False,
            subdim=0,
            imm2=BIG,
            shape=shape,
            row=get_dve_sub_opcode(op.name),
            isa_opcode=opc,
            ins=ins_l,
            outs=outs_l,
            perf_max=0,
        )
    )


_NC_CACHE = None


def _build():
    global _NC_CACHE
    if _NC_CACHE is not None:
        return _NC_CACHE
    nc = bacc.Bacc("TRN2", target_bir_lowering=False, debug=False, num_devices=8)
    x_blk = nc.dram_tensor("x_blk", [NB], F32, kind="ExternalInput")
    y_blk = nc.dram_tensor("y_blk", [NB], F32, kind="ExternalInput")
    x_full = nc.dram_tensor("x_full", [NF], F32, kind="ExternalInput")
    y_full = nc.dram_tensor("y_full", [NF], F32, kind="ExternalInput")
    mins0 = nc.dram_tensor("mins0", [P, T, 2], F32, kind="ExternalOutput")
    mins1 = nc.dram_tensor("mins1", [P, T], F32, kind="ExternalOutput")

    with tile.TileContext(nc) as tc:
        with (
            tc.tile_pool(name="bcast0", bufs=1) as bc_pool0,
            tc.tile_pool(name="bcast1", bufs=1) as bc_pool1,
            tc.tile_pool(name="small", bufs=1) as small,
        ):
            # row values: [128, T], partition p / tile t <- rows[p*T + t];
            # tiny strided loads ride the ACT ring so they don't serialize
            # ahead of the broadcasts on the SP ring
            rows0 = small.tile([P, T], F32, tag="rows0")
            nc.scalar.dma_start(rows0[:], x_blk.ap().rearrange("(p t) -> p t", p=P))
            rows1 = small.tile([P, T], F32, tag="rows1")
            nc.scalar.dma_start(rows1[:], y_blk.ap().rearrange("(p t) -> p t", p=P))

            # pre-issue every broadcast DMA on the SP HWDGE ring
            cols0 = []
            off = 0
            for ch, cw in enumerate(CHUNKS0):
                t0 = bc_pool0.tile([P, cw], F32, tag=f"c0_{ch}")
                nc.sync.dma_start(
                    t0[:],
                    y_full.ap()[off : off + cw].unsqueeze(0).partition_broadcast(P),
                )
                cols0.append(t0)
                off += cw
            cols1 = bc_pool1.tile([P, NF], F32, tag="c1")
            nc.sync.dma_start(
                cols1[:], x_full.ap().unsqueeze(0).partition_broadcast(P)
            )

            minw0 = small.tile([P, T, 2], F32, tag="minw0")
            minw1 = small.tile([P, T], F32, tag="minw1")

            for ch in range(len(CHUNKS0)):
                for p in range(NPAIR):
                    _emit(
                        nc.vector,
                        out=minw0[:, 2 * p : 2 * p + 2, ch],
                        in0=cols0[ch][:],
                        s0=rows0[:, 2 * p : 2 * p + 1],
                        s1=rows0[:, 2 * p + 1 : 2 * p + 2],
                    )
            for p in range(NPAIR):
                _emit(
                    nc.vector,
                    out=minw1[:, 2 * p : 2 * p + 2],
                    in0=cols1[:],
                    s0=rows1[:, 2 * p : 2 * p + 1],
                    s1=rows1[:, 2 * p + 1 : 2 * p + 2],
                )
            nc.sync.dma_start(mins0.ap(), minw0[:])
            nc.sync.dma_start(mins1.ap(), minw1[:])
    nc.compile()
    _NC_CACHE = nc
    return nc


def kernel(**inputs: np.ndarray) -> np.ndarray:
    x = np.ascontiguousarray(inputs["inputs"], dtype=np.float32).reshape(-1)
    y = np.ascontiguousarray(inputs["targets"], dtype=np.float32).reshape(-1)
    assert x.shape == (NF,) and y.shape == (NF,)

    nc = _build()
    in_maps = [
        {
            "x_blk": x[c * NB : (c + 1) * NB],
            "y_blk": y[c * NB : (c + 1) * NB],
            "x_full": x,
            "y_full": y,
        }
        for c in range(8)
    ]
    res = bass_utils.run_bass_kernel_spmd(nc, in_maps, core_ids=list(range(8)))

    cd_xy = 0.0
    cd_yx = 0.0
    for c in range(8):
        m0 = res.results[c]["mins0"]  # [P, T, 2] squared minima per chunk
        m1 = res.results[c]["mins1"]  # [P, T]
        cd_xy += np.sqrt(m0.min(axis=2)).sum(dtype=np.float64)
        cd_yx += np.sqrt(m1).sum(dtype=np.float64)
    val = ALPHA * cd_xy / NF + (1.0 - ALPHA) * cd_yx / NF
    return np.float32(val)
